# revision 14
# baseline (speedup 1.0000x reference)
"""Trainium2 Bass kernel for the GraphMAE-style GCN loss (nn_CG_30588757082611).

Strategy (8 NeuronCores, SPMD):
  - Nodes sharded 12500/core (padded to 12800); edges partitioned by dst owner.
  - Layer tables (h @ W, pre-scaled by src-degree norm) stored node-major fp16
    in DRAM; edge gathers via gpsimd.indirect_dma_start (128 rows/call).
  - Aggregation: one-hot matmuls (lhsT = gathered edge rows, rhs = iota==offset
    selection matrix) accumulated feature-major in PSUM per 512-dst bank.
  - BN statistics via per-feature free-dim reductions; BN1 stats all-reduced
    on-device (tiny collective); layer-2 hidden shards exchanged via AllGather.
  - The 17-node target branch mean is mathematically zero (BN over the same
    rows), so its normalized direction is pure f32 rounding noise; it is
    computed bit-identically to the reference with jax on CPU. The main branch
    (everything O(N·E)) runs on the NeuronCores.
"""

import sys

sys.path.insert(0, "/opt/trn_rl_repo")

import numpy as np

N = 100000
D = 128
C = 8
SH = 12500          # real nodes per core
SHP = 12800         # padded nodes per core
NT = C * SHP        # padded table rows (102400)
W = 64              # dst window width (S matrix / psum slice)
NWIN = SHP // W     # 200 windows per core
VWIN = 195          # window holding the virtual mask columns
SWW = 512           # psum bank width (8 windows)
NSW = SHP // SWW    # 25 super windows
WGRP = 32           # windows per psum group
NGRP = (NWIN + WGRP - 1) // WGRP  # 7 groups (last short)
NM = 17
BN_EPS = 1e-5
SENT_OFF = np.float16(100.0)

_cached = {}


def _pad_id(n):
    return (n // SH) * SHP + (n % SH)


def _prep_layer_edges(src_pad, dst_core, dst_local, extra_src_pad=None, extra_off=None):
    """Per-core gather/matmul schedule for one layer (no index chunking;
    indirect DMA uses int32 indices). Tiles of 128 edges, window-pure; per
    (core, window) cell padded to the max tile count over cores."""
    cell_src = [[None] * NWIN for _ in range(C)]
    cell_off = [[None] * NWIN for _ in range(C)]
    order = np.lexsort((src_pad, dst_local, dst_core))
    s_s, s_c, s_l = src_pad[order], dst_core[order], dst_local[order]
    s_w = s_l // W
    s_o = (s_l % W).astype(np.float16)
    key = s_c.astype(np.int64) * NWIN + s_w
    uniq, starts = np.unique(key, return_index=True)
    starts = list(starts) + [len(key)]
    for i, k in enumerate(uniq):
        wd = int(k % NWIN)
        co = int(k // NWIN)
        a, b = starts[i], starts[i + 1]
        cell_src[co][wd] = s_s[a:b]
        cell_off[co][wd] = s_o[a:b]
    if extra_src_pad is not None:
        for co in range(C):
            wd = VWIN
            cs = cell_src[co][wd]
            cell_src[co][wd] = (extra_src_pad if cs is None
                                else np.concatenate([cs, extra_src_pad]))
            cc = cell_off[co][wd]
            eo = extra_off.astype(np.float16)
            cell_off[co][wd] = eo if cc is None else np.concatenate([cc, eo])

    tiles = []          # per tile: window id
    idx_cols = [[] for _ in range(C)]
    offs_cols = [[] for _ in range(C)]
    for wd in range(NWIN):
        cnts = [0 if cell_src[co][wd] is None else len(cell_src[co][wd])
                for co in range(C)]
        nt = (max(cnts) + 127) // 128
        for co in range(C):
            ns = nt * 128
            sp = np.zeros(ns, np.int64)
            op = np.full(ns, SENT_OFF, np.float16)
            if cell_src[co][wd] is not None:
                cs = cell_src[co][wd]
                sp[: len(cs)] = cs
                op[: len(cs)] = cell_off[co][wd]
            idx_cols[co].append(sp.astype(np.int32))
            offs_cols[co].append(op)
        tiles.extend([wd] * nt)
    total_tiles = len(tiles)
    idx32 = []
    offs = []
    for co in range(C):
        flat = np.concatenate(idx_cols[co]).reshape(total_tiles, 128).T
        idx32.append(np.ascontiguousarray(flat))               # [128, T]
        ob = np.concatenate(offs_cols[co]).reshape(total_tiles, 128).T
        offs.append(np.ascontiguousarray(ob))                  # [128, T]
    return dict(tiles=tiles, idx32=idx32, offs=offs, total_tiles=total_tiles)


def _host_prep(inputs):
    feat = np.asarray(inputs["feat"], np.float32)
    src = np.asarray(inputs["src"], np.int64)
    dst = np.asarray(inputs["dst"], np.int64)
    mask = np.asarray(inputs["mask_nodes"], np.int64)

    deg_out = np.bincount(src, minlength=N).astype(np.float32)
    deg_in = np.bincount(dst, minlength=N).astype(np.float32)
    dns = np.where(deg_out > 0, deg_out ** -0.5, 0.0).astype(np.float32)
    dnd = np.where(deg_in > 0, deg_in ** -0.5, 0.0).astype(np.float32)

    src_pad = _pad_id(src)
    dst_core = dst // SH
    dst_local = dst % SH

    l1 = _prep_layer_edges(src_pad, dst_core, dst_local)
    # layer 2 adds replicated "virtual" copies of mask-node in-edges, landing at
    # window 195 offsets 20..36 (local cols 12500..12516)
    vsrc, voff = [], []
    for j, m in enumerate(mask):
        sel = dst == m
        vsrc.append(src_pad[sel])
        voff.append(np.full(sel.sum(), 20 + j, np.float16))
    vsrc = np.concatenate(vsrc)
    voff = np.concatenate(voff)
    l2 = _prep_layer_edges(src_pad, dst_core, dst_local, vsrc, voff)

    # feature-major padded masked features (fp16), replicated to all cores
    featm = feat.copy()
    featm[mask] = 0.0
    feat_fm = np.zeros((D, NT), np.float16)
    for c in range(C):
        feat_fm[:, c * SHP: c * SHP + SH] = featm[c * SH:(c + 1) * SH].T
    # node-major dn_src per production tile (full padded table), [128, NT//128]
    dns_pad = np.zeros(NT, np.float32)
    for c in range(C):
        dns_pad[c * SHP: c * SHP + SH] = dns[c * SH:(c + 1) * SH]
    dn_nm = np.ascontiguousarray(dns_pad.reshape(NT // 128, 128).T)  # [128, 800]
    # per-core shard dn_src node-major [128, 100]
    dn_sh = []
    for c in range(C):
        v = dns_pad[c * SHP:(c + 1) * SHP]
        dn_sh.append(np.ascontiguousarray(v.reshape(SHP // 128, 128).T))
    # per-core dst-degree norm replicated across partitions [128, SHP] fp16,
    # with virtual columns = mask nodes' dnd
    dnd_rep = []
    for c in range(C):
        v = np.zeros(SHP, np.float32)
        v[:SH] = dnd[c * SH:(c + 1) * SH]
        v[SH: SH + NM] = dnd[mask]
        dnd_rep.append(np.tile(v.astype(np.float16)[None, :], (128, 1)))
    return dict(l1=l1, l2=l2, feat_fm=feat_fm, dn_nm=dn_nm, dn_sh=dn_sh,
                dnd_rep=dnd_rep)


def _build(prep, w_np):
    import concourse.bass as bass
    import concourse.mybir as mybir
    import concourse.tile as tile

    dt = mybir.dt
    AF = mybir.ActivationFunctionType
    OP = mybir.AluOpType
    l1, l2 = prep["l1"], prep["l2"]
    a1 = float(w_np["a1o"])

    nc = bass.Bass()
    t_feat = nc.declare_dram_parameter("feat_fm", [D, NT], dt.float16, isOutput=False)
    t_w1 = nc.declare_dram_parameter("w1", [D, D], dt.float16, isOutput=False)
    t_w2 = nc.declare_dram_parameter("w2", [D, D], dt.float16, isOutput=False)
    t_dnnm = nc.declare_dram_parameter("dn_nm", [128, NT // 128], dt.float32, isOutput=False)
    t_dnsh = nc.declare_dram_parameter("dn_sh", [128, SHP // 128], dt.float32, isOutput=False)
    t_dnd = nc.declare_dram_parameter("dnd_rep", [128, SHP], dt.float16, isOutput=False)
    t_g1 = nc.declare_dram_parameter("g1", [128, 1], dt.float32, isOutput=False)
    t_be1 = nc.declare_dram_parameter("be1", [128, 1], dt.float32, isOutput=False)
    t_idx1 = nc.declare_dram_parameter("idx1", [128, l1["total_tiles"]], dt.int32, isOutput=False)
    t_idx2 = nc.declare_dram_parameter("idx2", [128, l2["total_tiles"]], dt.int32, isOutput=False)
    t_off1 = nc.declare_dram_parameter("off1", [128, l1["total_tiles"]], dt.float16, isOutput=False)
    t_off2 = nc.declare_dram_parameter("off2", [128, l2["total_tiles"]], dt.float16, isOutput=False)
    t_tail = nc.declare_dram_parameter("tail", [128, NM + 2], dt.float32, isOutput=True)

    table1 = nc.dram_tensor("table1", [NT, D], dt.float16)
    bounce = nc.dram_tensor("bounce", [SHP, D], dt.float16)
    table2 = nc.dram_tensor("table2", [NT, D], dt.float16, addr_space="Shared")
    st_in = nc.dram_tensor("st_in", [128, 2], dt.float32)
    st_out = nc.dram_tensor("st_out", [128, 2], dt.float32, addr_space="Shared")

    PROD_B = 8  # production tiles per staged table write

    with tile.TileContext(nc) as tc:
        with (
            tc.tile_pool(name="const", bufs=1) as cpool,
            tc.tile_pool(name="ld", bufs=2) as lpool,
            tc.tile_pool(name="gath", bufs=3) as gpool,
            tc.tile_pool(name="idxp", bufs=3) as ipool,
            tc.tile_pool(name="sp", bufs=6) as spool,
            tc.tile_pool(name="feat", bufs=3) as fpool,
            tc.tile_pool(name="stage", bufs=3) as stpool,
            tc.tile_pool(name="psum", bufs=4, space="PSUM") as ppool,
            tc.tile_pool(name="psmm", bufs=2, space="PSUM") as pmpool,
            tc.tile_pool(name="y1p", bufs=1) as ypool,
            tc.tile_pool(name="x2p", bufs=1) as xpool,
            tc.tile_pool(name="zt", bufs=3) as zpool,
            tc.tile_pool(name="stats", bufs=1) as stats_pool,
        ):
            # ---- constants (DVE-local copies so consumers carry <=1 wait) ----
            def dve_const(t_dram, shape, dtp):
                nm = t_dram.name if hasattr(t_dram, "name") else str(t_dram)
                ld = lpool.tile(shape, dtp, tag="ld")
                nc.sync.dma_start(out=ld[:], in_=t_dram[:])
                cp = cpool.tile(shape, dtp, tag=f"const_{nm}")
                nc.vector.tensor_copy(out=cp[:], in_=ld[:])
                return cp

            iota_g = cpool.tile([128, W], dt.float16)
            nc.gpsimd.iota(iota_g[:], pattern=[[1, W]], channel_multiplier=0,
                           allow_small_or_imprecise_dtypes=True)
            iota_sb = cpool.tile([128, W], dt.float16)
            nc.vector.tensor_copy(out=iota_sb[:], in_=iota_g[:])

            w1_sb = dve_const(t_w1, [D, D], dt.float16)
            w2_sb = dve_const(t_w2, [D, D], dt.float16)
            dnnm_sb = dve_const(t_dnnm, [128, NT // 128], dt.float32)
            dnsh_sb = dve_const(t_dnsh, [128, SHP // 128], dt.float32)
            dnd_sb = dve_const(t_dnd, [128, SHP], dt.float16)
            g1_sb = dve_const(t_g1, [128, 1], dt.float32)
            be1_sb = dve_const(t_be1, [128, 1], dt.float32)
            off1_sb = dve_const(t_off1, [128, l1["total_tiles"]], dt.float16)
            off2_sb = dve_const(t_off2, [128, l2["total_tiles"]], dt.float16)
            idx1_sb = dve_const(t_idx1, [128, l1["total_tiles"]], dt.int32)
            idx2_sb = dve_const(t_idx2, [128, l2["total_tiles"]], dt.int32)

            y1_sb = ypool.tile([128, SHP], dt.float32)
            x2_sb = xpool.tile([128, SHP], dt.float16)
            spa1 = stats_pool.tile([128, NSW], dt.float32)   # L1 sum partials
            spb1 = stats_pool.tile([128, NSW], dt.float32)   # L1 sumsq partials
            spa2 = stats_pool.tile([128, NSW], dt.float32)   # L2 sum partials
            spb2 = stats_pool.tile([128, NSW], dt.float32)   # L2 sumsq partials
            tail_sb = stats_pool.tile([128, NM + 2], dt.float32)

            # ================= L1: full table production =================
            # h1n[node, :] = dn_src[node] * (feat_fm[:, node].T @ W1)
            nprod = NT // 128
            for b0 in range(0, nprod, PROD_B):
                bts = min(PROD_B, nprod - b0)
                ft = fpool.tile([128, PROD_B * 128], dt.float16, tag="ft")
                nc.sync.dma_start(out=ft[:, : bts * 128],
                                  in_=t_feat[:, b0 * 128:(b0 + bts) * 128])
                stg = stpool.tile([128, PROD_B * 128], dt.float16, tag="prod_stage")
                for t in range(bts):
                    ti = b0 + t
                    ps = pmpool.tile([128, 128], dt.float32, space="PSUM", tag="prod_ps")
                    nc.tensor.matmul(out=ps[:], lhsT=ft[:, t * 128:(t + 1) * 128],
                                     rhs=w1_sb[:], start=True, stop=True)
                    nc.vector.tensor_tensor(
                        out=stg[:, t * 128:(t + 1) * 128], in0=ps[:],
                        in1=dnnm_sb[:, ti:ti + 1].to_broadcast([128, 128]),
                        op=OP.mult)
                nc.sync.dma_start(
                    out=table1[b0 * 128:(b0 + bts) * 128, :].rearrange(
                        "(b p) d -> p b d", p=128),
                    in_=stg[:, : bts * 128].rearrange("p (b d) -> p b d", d=D))

            # ================= aggregation pass helper =================
            def agg_layer(lx, idx_sb, off_sb, table_t, out_cb):
                tiles = lx["tiles"]
                T = lx["total_tiles"]
                t = 0
                for g in range(NGRP):
                    w0, w1_ = g * WGRP, min((g + 1) * WGRP, NWIN)
                    nbank = (w1_ - w0 + 7) // 8
                    banks = []
                    for bi in range(nbank):
                        ps = ppool.tile([128, SWW], dt.float32, space="PSUM",
                                        tag="agg_ps")
                        nc.vector.memset(ps[:], 0.0)
                        banks.append(ps)
                    last_wd = -1
                    while t < T and tiles[t] < w1_:
                        wd = tiles[t]
                        gt = gpool.tile([128, D], dt.float16, tag="gt")
                        nc.gpsimd.indirect_dma_start(
                            out=gt[:], out_offset=None,
                            in_=table_t[:],
                            in_offset=bass.IndirectOffsetOnAxis(
                                ap=idx_sb[:, t:t + 1], axis=0))
                        st = spool.tile([128, W], dt.float16, tag="s")
                        nc.vector.tensor_tensor(
                            out=st[:], in0=iota_sb[:],
                            in1=off_sb[:, t:t + 1].to_broadcast([128, W]),
                            op=OP.is_equal)
                        bi = (wd - w0) // 8
                        co = ((wd - w0) % 8) * W
                        nc.tensor.matmul(
                            out=banks[bi][:, co:co + W],
                            lhsT=gt[:], rhs=st[:], start=(wd != last_wd),
                            stop=False)
                        last_wd = wd
                        t += 1
                    for bi in range(nbank):
                        out_cb((w0 // 8) + bi, banks[bi])

            # ---- L1 aggregation: y1 = psum * dnd ; stats partials ----
            def l1_out(sw, ps):
                sl = slice(sw * SWW, (sw + 1) * SWW)
                nc.vector.tensor_tensor(out=y1_sb[:, sl], in0=ps[:],
                                        in1=dnd_sb[:, sl], op=OP.mult)
                lim = SWW if sw < NSW - 1 else (SH - (NSW - 1) * SWW)
                nc.vector.reduce_sum(out=spa1[:, sw:sw + 1],
                                     in_=y1_sb[:, sw * SWW: sw * SWW + lim],
                                     axis=mybir.AxisListType.X)
                sq = zpool.tile([128, SWW], dt.float32, tag="sq")
                nc.scalar.activation(out=sq[:, :lim],
                                     in_=y1_sb[:, sw * SWW: sw * SWW + lim],
                                     func=AF.Square,
                                     accum_out=spb1[:, sw:sw + 1])

            agg_layer(l1, idx1_sb, off1_sb, table1, l1_out)

            # ---- BN1 stats allreduce ----
            stat_c = stats_pool.tile([128, 2], dt.float32)
            nc.vector.reduce_sum(out=stat_c[:, 0:1], in_=spa1[:],
                                 axis=mybir.AxisListType.X)
            nc.vector.reduce_sum(out=stat_c[:, 1:2], in_=spb1[:],
                                 axis=mybir.AxisListType.X)
            nc.sync.dma_start(out=st_in[:], in_=stat_c[:])
            nc.gpsimd.collective_compute(
                "AllReduce", mybir.AluOpType.add,
                replica_groups=[list(range(C))],
                ins=[st_in[:]], outs=[st_out[:]],
            )
            stat_ld = lpool.tile([128, 2], dt.float32, tag="statld")
            nc.sync.dma_start(out=stat_ld[:], in_=st_out[:])
            stat_t = stats_pool.tile([128, 2], dt.float32)
            nc.vector.tensor_copy(out=stat_t[:], in_=stat_ld[:])

            # mu = sum/N ; var = sumsq/N - mu^2 ; s = g1 / sqrt(var+eps)
            # t = be1 - mu * s
            mu = stats_pool.tile([128, 1], dt.float32)
            nc.vector.tensor_scalar_mul(mu[:], stat_t[:, 0:1], 1.0 / N)
            var = stats_pool.tile([128, 1], dt.float32)
            nc.vector.tensor_scalar_mul(var[:], stat_t[:, 1:2], 1.0 / N)
            mu2 = stats_pool.tile([128, 1], dt.float32)
            nc.vector.tensor_tensor(out=mu2[:], in0=mu[:], in1=mu[:], op=OP.mult)
            nc.vector.tensor_tensor(out=var[:], in0=var[:], in1=mu2[:], op=OP.subtract)
            nc.vector.tensor_scalar_add(var[:], var[:], BN_EPS)
            sd = stats_pool.tile([128, 1], dt.float32)
            nc.scalar.activation(out=sd[:], in_=var[:], func=AF.Sqrt)
            rsd = stats_pool.tile([128, 1], dt.float32)
            nc.vector.reciprocal(out=rsd[:], in_=sd[:])
            # one Newton step on r ~= 1/sqrt(v): r <- r*(1.5 - 0.5*v*r^2)
            nt1 = stats_pool.tile([128, 1], dt.float32)
            nc.vector.tensor_tensor(out=nt1[:], in0=rsd[:], in1=rsd[:], op=OP.mult)
            nc.vector.tensor_tensor(out=nt1[:], in0=nt1[:], in1=var[:], op=OP.mult)
            nc.vector.tensor_scalar(out=nt1[:], in0=nt1[:], scalar1=-0.5,
                                    scalar2=1.5, op0=OP.mult, op1=OP.add)
            nc.vector.tensor_tensor(out=rsd[:], in0=rsd[:], in1=nt1[:], op=OP.mult)
            bs = stats_pool.tile([128, 1], dt.float32)
            nc.vector.tensor_tensor(out=bs[:], in0=g1_sb[:], in1=rsd[:], op=OP.mult)
            bt = stats_pool.tile([128, 1], dt.float32)
            nc.vector.tensor_tensor(out=bt[:], in0=mu[:], in1=bs[:], op=OP.mult)
            nc.vector.tensor_tensor(out=bt[:], in0=be1_sb[:], in1=bt[:], op=OP.subtract)

            # ---- x2 = prelu(y1 * s + t) via ACT LeakyRelu ----
            for sw in range(NSW):
                sl = slice(sw * SWW, (sw + 1) * SWW)
                nc.scalar.activation(out=x2_sb[:, sl], in_=y1_sb[:, sl],
                                     func=AF.Lrelu, scale=bs[:, 0:1],
                                     bias=bt[:, 0:1], alpha=a1)

            # ---- L2 shard table production + exchange ----
            nprod2 = SHP // 128
            for b0 in range(0, nprod2, PROD_B):
                bts = min(PROD_B, nprod2 - b0)
                stg = stpool.tile([128, PROD_B * 128], dt.float16, tag="prod_stage")
                for t in range(bts):
                    ti = b0 + t
                    ps = pmpool.tile([128, 128], dt.float32, space="PSUM", tag="prod_ps")
                    nc.tensor.matmul(out=ps[:], lhsT=x2_sb[:, ti * 128:(ti + 1) * 128],
                                     rhs=w2_sb[:], start=True, stop=True)
                    nc.vector.tensor_tensor(
                        out=stg[:, t * 128:(t + 1) * 128], in0=ps[:],
                        in1=dnsh_sb[:, ti:ti + 1].to_broadcast([128, 128]),
                        op=OP.mult)
                nc.sync.dma_start(
                    out=bounce[b0 * 128:(b0 + bts) * 128, :].rearrange(
                        "(b p) d -> p b d", p=128),
                    in_=stg[:, : bts * 128].rearrange("p (b d) -> p b d", d=D))
            nc.gpsimd.collective_compute(
                "AllGather", mybir.AluOpType.bypass,
                replica_groups=[list(range(C))],
                ins=[bounce[:]], outs=[table2[:]],
            )

            # ---- L2 aggregation: stats partials + mask columns only ----
            def l2_out(sw, ps):
                z = zpool.tile([128, SWW], dt.float32, tag="z")
                sl = slice(sw * SWW, (sw + 1) * SWW)
                nc.vector.tensor_tensor(out=z[:], in0=ps[:], in1=dnd_sb[:, sl],
                                        op=OP.mult)
                lim = SWW if sw < NSW - 1 else (SH - (NSW - 1) * SWW)
                nc.vector.reduce_sum(out=spa2[:, sw:sw + 1], in_=z[:, :lim],
                                     axis=mybir.AxisListType.X)
                sq = zpool.tile([128, SWW], dt.float32, tag="sq")
                nc.scalar.activation(out=sq[:, :lim], in_=z[:, :lim],
                                     func=AF.Square,
                                     accum_out=spb2[:, sw:sw + 1])
                if sw == NSW - 1:
                    nc.vector.tensor_copy(out=tail_sb[:, 0:NM],
                                          in_=z[:, lim: lim + NM])

            agg_layer(l2, idx2_sb, off2_sb, table2, l2_out)

            nc.vector.reduce_sum(out=tail_sb[:, NM:NM + 1], in_=spa2[:],
                                 axis=mybir.AxisListType.X)
            nc.vector.reduce_sum(out=tail_sb[:, NM + 1:NM + 2], in_=spb2[:],
                                 axis=mybir.AxisListType.X)
            nc.sync.dma_start(out=t_tail[:], in_=tail_sb[:])

    return nc


def _legalize_waits(nc):
    """This toolchain's walrus accepts only ONE sync-wait command per
    instruction; Tile emits up to ~3. Split excess waits onto same-engine
    NoOps placed immediately before (engine program order preserves
    semantics)."""
    import concourse.mybir as mybir
    f = nc.m.functions[0]
    k = 0
    for b in f.blocks:
        il = b.instructions
        out = []
        changed = False
        for inst in il:
            si = inst.sync_info
            nw = list(si.on_wait) if si is not None else []
            if len(nw) > 1:
                for w in nw[:-1]:
                    nop = mybir.InstNoOp(name=f"I-wfix-{k}", ins=[], outs=[])
                    k += 1
                    nop.engine = inst.engine
                    nop.sync_info = mybir.SyncInfo(on_wait=[w], on_update=[])
                    out.append(nop)
                si.on_wait = nw[-1:]
                changed = True
            out.append(inst)
        if changed:
            b.instructions = out
    return k


def _host_target_and_loss(inputs, tail_results):
    """Target branch + final loss, replicating the reference's jax-CPU f32
    arithmetic exactly (the target mean is BN-forced to ~0, so its direction is
    rounding noise that must be reproduced bit-for-bit)."""
    import jax
    cpu = jax.devices("cpu")[0]
    import jax.numpy as jnp

    with jax.default_device(cpu):
        def _deg_norm(idx, n):
            deg = jax.ops.segment_sum(jnp.ones(idx.shape[0], jnp.float32), idx,
                                      num_segments=n)
            return jnp.where(deg > 0, deg ** -0.5, 0.0)

        def graph_conv(x, Wm, b, s_, d_, n):
            h = x @ Wm
            h = h * _deg_norm(s_, n)[:, None]
            agg = jax.ops.segment_sum(h[s_], d_, num_segments=n)
            return agg * _deg_norm(d_, n)[:, None] + b

        def batch_norm(x, g, be):
            mu = x.mean(axis=0)
            var = ((x - mu) ** 2).mean(axis=0)
            return (x - mu) * jax.lax.rsqrt(var + BN_EPS) * g + be

        def prelu(x, a):
            return jnp.where(x >= 0, x, a * x)

        def encoder(x, s_, d_, n, p):
            (W1, b1, g1, be1, a1, W2, b2, g2, be2, a2) = p
            h = graph_conv(x, W1, b1, s_, d_, n)
            h = prelu(batch_norm(h, g1, be1), a1)
            h = graph_conv(h, W2, b2, s_, d_, n)
            h = batch_norm(h, g2, be2)
            return h

        def sce_loss(x, y):
            xn = x / jnp.maximum(jnp.linalg.norm(x), 1e-12)
            yn = y / jnp.maximum(jnp.linalg.norm(y), 1e-12)
            return 1.0 - jnp.sum(xn * yn)

        feat = jnp.asarray(inputs["feat"], jnp.float32)
        mask = jnp.asarray(inputs["mask_nodes"], jnp.int32)
        pt = tuple(jnp.asarray(inputs[nm + "t"]) for nm in
                   ["W1", "b1", "g1", "be1", "a1", "W2", "b2", "g2", "be2", "a2"])
        sub_feat = feat[mask]
        h2 = encoder(sub_feat, jnp.asarray(inputs["sub_src"], jnp.int32),
                     jnp.asarray(inputs["sub_dst"], jnp.int32), NM, pt)
        bvec = h2.mean(axis=0)

        # combine device partials -> a vector
        sums = np.zeros((128,), np.float32)
        sumsq = np.zeros((128,), np.float32)
        for r in tail_results:
            sums += r[:, NM]
            sumsq += r[:, NM + 1]
        mu2 = sums / np.float32(N)
        var2 = sumsq / np.float32(N) - mu2 * mu2
        g2 = np.asarray(inputs["g2o"], np.float32)
        be2 = np.asarray(inputs["be2o"], np.float32)
        s2 = g2 / np.sqrt(var2 + np.float32(BN_EPS))
        cols = tail_results[0][:, :NM]           # [128 feat, 17]
        hbn = (cols - mu2[:, None]) * s2[:, None] + be2[:, None]
        avec = hbn.mean(axis=1)

        loss = sce_loss(jnp.asarray(avec), bvec)
        return np.asarray(loss, np.float32)


def kernel(**inputs):
    from concourse.bass_utils import run_bass_kernel_spmd

    prep = _host_prep(inputs)
    nc = _build(prep, inputs)

    w1 = np.ascontiguousarray(np.asarray(inputs["W1o"], np.float32).astype(np.float16))
    w2 = np.ascontiguousarray(np.asarray(inputs["W2o"], np.float32).astype(np.float16))
    g1 = np.asarray(inputs["g1o"], np.float32).reshape(128, 1)
    be1 = np.asarray(inputs["be1o"], np.float32).reshape(128, 1)

    in_maps = []
    for c in range(C):
        in_maps.append({
            "feat_fm": prep["feat_fm"],
            "w1": w1, "w2": w2,
            "dn_nm": prep["dn_nm"],
            "dn_sh": prep["dn_sh"][c],
            "dnd_rep": prep["dnd_rep"][c],
            "g1": g1, "be1": be1,
            "idx1": prep["l1"]["idx32"][c],
            "idx2": prep["l2"]["idx32"][c],
            "off1": prep["l1"]["offs"][c],
            "off2": prep["l2"]["offs"][c],
        })
    _legalize_waits(nc)
    res = run_bass_kernel_spmd(nc, in_maps, core_ids=list(range(C)))
    tails = [res.results[c]["tail"] for c in range(C)]
    return _host_target_and_loss(inputs, tails)


# revision 17
# speedup vs baseline: 1.0669x; 1.0669x over previous
"""Trainium2 Bass kernel for the GraphMAE-style GCN loss (nn_CG_30588757082611).

Strategy (8 NeuronCores, SPMD):
  - Nodes sharded 12500/core (padded to 12800); edges partitioned by dst owner.
  - Layer tables (h @ W, pre-scaled by src-degree norm) stored node-major fp16
    in DRAM; edge gathers via gpsimd.indirect_dma_start (128 rows/call).
  - Aggregation: one-hot matmuls (lhsT = gathered edge rows, rhs = iota==offset
    selection matrix) accumulated feature-major in PSUM per 512-dst bank.
  - BN statistics via per-feature free-dim reductions; BN1 stats all-reduced
    on-device (tiny collective); layer-2 hidden shards exchanged via AllGather.
  - The 17-node target branch mean is mathematically zero (BN over the same
    rows), so its normalized direction is pure f32 rounding noise; it is
    computed bit-identically to the reference with jax on CPU. The main branch
    (everything O(N·E)) runs on the NeuronCores.
"""

import sys

sys.path.insert(0, "/opt/trn_rl_repo")

import numpy as np

N = 100000
D = 128
C = 8
SH = 12500          # real nodes per core
SHP = 12800         # padded nodes per core
NT = C * SHP        # padded table rows (102400)
W = 64              # dst window width (S matrix / psum slice)
NWIN = SHP // W     # 200 windows per core
VWIN = 195          # window holding the virtual mask columns
SWW = 512           # psum bank width (8 windows)
NSW = SHP // SWW    # 25 super windows
WGRP = 32           # windows per psum group
NGRP = (NWIN + WGRP - 1) // WGRP  # 7 groups (last short)
NM = 17
BN_EPS = 1e-5
SENT_OFF = np.float16(100.0)

_cached = {}


def _pad_id(n):
    return (n // SH) * SHP + (n % SH)


def _prep_layer_edges(src_pad, dst_core, dst_local, extra_src_pad=None, extra_off=None):
    """Per-core gather/matmul schedule for one layer (no index chunking;
    indirect DMA uses int32 indices). Tiles of 128 edges, window-pure; per
    (core, window) cell padded to the max tile count over cores."""
    cell_src = [[None] * NWIN for _ in range(C)]
    cell_off = [[None] * NWIN for _ in range(C)]
    order = np.lexsort((src_pad, dst_local, dst_core))
    s_s, s_c, s_l = src_pad[order], dst_core[order], dst_local[order]
    s_w = s_l // W
    s_o = (s_l % W).astype(np.float16)
    key = s_c.astype(np.int64) * NWIN + s_w
    uniq, starts = np.unique(key, return_index=True)
    starts = list(starts) + [len(key)]
    for i, k in enumerate(uniq):
        wd = int(k % NWIN)
        co = int(k // NWIN)
        a, b = starts[i], starts[i + 1]
        cell_src[co][wd] = s_s[a:b]
        cell_off[co][wd] = s_o[a:b]
    if extra_src_pad is not None:
        for co in range(C):
            wd = VWIN
            cs = cell_src[co][wd]
            cell_src[co][wd] = (extra_src_pad if cs is None
                                else np.concatenate([cs, extra_src_pad]))
            cc = cell_off[co][wd]
            eo = extra_off.astype(np.float16)
            cell_off[co][wd] = eo if cc is None else np.concatenate([cc, eo])

    tiles = []          # per tile: window id
    idx_cols = [[] for _ in range(C)]
    offs_cols = [[] for _ in range(C)]
    for wd in range(NWIN):
        cnts = [0 if cell_src[co][wd] is None else len(cell_src[co][wd])
                for co in range(C)]
        nt = (max(cnts) + 127) // 128
        for co in range(C):
            ns = nt * 128
            sp = np.zeros(ns, np.int64)
            op = np.full(ns, SENT_OFF, np.float16)
            if cell_src[co][wd] is not None:
                cs = cell_src[co][wd]
                sp[: len(cs)] = cs
                op[: len(cs)] = cell_off[co][wd]
            idx_cols[co].append(sp.astype(np.int32))
            offs_cols[co].append(op)
        tiles.extend([wd] * nt)
    total_tiles = len(tiles)
    idx32 = []
    offs = []
    for co in range(C):
        flat = np.concatenate(idx_cols[co]).reshape(total_tiles, 128).T
        idx32.append(np.ascontiguousarray(flat))               # [128, T]
        ob = np.concatenate(offs_cols[co]).reshape(total_tiles, 128).T
        offs.append(np.ascontiguousarray(ob))                  # [128, T]
    return dict(tiles=tiles, idx32=idx32, offs=offs, total_tiles=total_tiles)


def _host_prep(inputs):
    feat = np.asarray(inputs["feat"], np.float32)
    src = np.asarray(inputs["src"], np.int64)
    dst = np.asarray(inputs["dst"], np.int64)
    mask = np.asarray(inputs["mask_nodes"], np.int64)

    deg_out = np.bincount(src, minlength=N).astype(np.float32)
    deg_in = np.bincount(dst, minlength=N).astype(np.float32)
    dns = np.where(deg_out > 0, deg_out ** -0.5, 0.0).astype(np.float32)
    dnd = np.where(deg_in > 0, deg_in ** -0.5, 0.0).astype(np.float32)

    src_pad = _pad_id(src)
    dst_core = dst // SH
    dst_local = dst % SH

    l1 = _prep_layer_edges(src_pad, dst_core, dst_local)
    # layer 2 adds replicated "virtual" copies of mask-node in-edges, landing at
    # window 195 offsets 20..36 (local cols 12500..12516)
    vsrc, voff = [], []
    for j, m in enumerate(mask):
        sel = dst == m
        vsrc.append(src_pad[sel])
        voff.append(np.full(sel.sum(), 20 + j, np.float16))
    vsrc = np.concatenate(vsrc)
    voff = np.concatenate(voff)
    l2 = _prep_layer_edges(src_pad, dst_core, dst_local, vsrc, voff)

    # feature-major padded masked features (fp16), replicated to all cores
    featm = feat.copy()
    featm[mask] = 0.0
    feat_fm = np.zeros((D, NT), np.float16)
    for c in range(C):
        feat_fm[:, c * SHP: c * SHP + SH] = featm[c * SH:(c + 1) * SH].T
    # node-major dn_src per production tile (full padded table), [128, NT//128]
    dns_pad = np.zeros(NT, np.float32)
    for c in range(C):
        dns_pad[c * SHP: c * SHP + SH] = dns[c * SH:(c + 1) * SH]
    dn_nm = np.ascontiguousarray(dns_pad.reshape(NT // 128, 128).T)  # [128, 800]
    # per-core shard dn_src node-major [128, 100]
    dn_sh = []
    for c in range(C):
        v = dns_pad[c * SHP:(c + 1) * SHP]
        dn_sh.append(np.ascontiguousarray(v.reshape(SHP // 128, 128).T))
    # per-core dst-degree norm replicated across partitions [128, SHP] fp16,
    # with virtual columns = mask nodes' dnd
    dnd_rep = []
    for c in range(C):
        v = np.zeros(SHP, np.float32)
        v[:SH] = dnd[c * SH:(c + 1) * SH]
        v[SH: SH + NM] = dnd[mask]
        dnd_rep.append(np.tile(v.astype(np.float16)[None, :], (128, 1)))
    return dict(l1=l1, l2=l2, feat_fm=feat_fm, dn_nm=dn_nm, dn_sh=dn_sh,
                dnd_rep=dnd_rep)


def _build(prep, w_np):
    import concourse.bass as bass
    import concourse.mybir as mybir
    import concourse.tile as tile

    dt = mybir.dt
    AF = mybir.ActivationFunctionType
    OP = mybir.AluOpType
    l1, l2 = prep["l1"], prep["l2"]
    a1 = float(w_np["a1o"])

    nc = bass.Bass()
    t_feat = nc.declare_dram_parameter("feat_fm", [D, NT], dt.float16, isOutput=False)
    t_w1 = nc.declare_dram_parameter("w1", [D, D], dt.float16, isOutput=False)
    t_w2 = nc.declare_dram_parameter("w2", [D, D], dt.float16, isOutput=False)
    t_dnnm = nc.declare_dram_parameter("dn_nm", [128, NT // 128], dt.float32, isOutput=False)
    t_dnsh = nc.declare_dram_parameter("dn_sh", [128, SHP // 128], dt.float32, isOutput=False)
    t_dnd = nc.declare_dram_parameter("dnd_rep", [128, SHP], dt.float16, isOutput=False)
    t_g1 = nc.declare_dram_parameter("g1", [128, 1], dt.float32, isOutput=False)
    t_be1 = nc.declare_dram_parameter("be1", [128, 1], dt.float32, isOutput=False)
    t_idx1 = nc.declare_dram_parameter("idx1", [128, l1["total_tiles"]], dt.int32, isOutput=False)
    t_idx2 = nc.declare_dram_parameter("idx2", [128, l2["total_tiles"]], dt.int32, isOutput=False)
    t_off1 = nc.declare_dram_parameter("off1", [128, l1["total_tiles"]], dt.float16, isOutput=False)
    t_off2 = nc.declare_dram_parameter("off2", [128, l2["total_tiles"]], dt.float16, isOutput=False)
    t_tail = nc.declare_dram_parameter("tail", [128, NM + 2], dt.float32, isOutput=True)

    table1 = nc.dram_tensor("table1", [NT, D], dt.float16)
    bounce = nc.dram_tensor("bounce", [SHP, D], dt.float16)
    table2 = nc.dram_tensor("table2", [NT, D], dt.float16, addr_space="Shared")
    st_in = nc.dram_tensor("st_in", [128, 2], dt.float32)
    st_out = nc.dram_tensor("st_out", [128, 2], dt.float32, addr_space="Shared")

    PROD_B = 16  # production tiles per staged table write

    with tile.TileContext(nc) as tc:
        with (
            tc.tile_pool(name="const", bufs=1) as cpool,
            tc.tile_pool(name="ld", bufs=2) as lpool,
            tc.tile_pool(name="gath", bufs=10) as gpool,
            tc.tile_pool(name="idxp", bufs=3) as ipool,
            tc.tile_pool(name="sp", bufs=8) as spool,
            tc.tile_pool(name="feat", bufs=3) as fpool,
            tc.tile_pool(name="stage", bufs=2) as stpool,
            tc.tile_pool(name="psum", bufs=4, space="PSUM") as ppool,
            tc.tile_pool(name="psmm", bufs=2, space="PSUM") as pmpool,
            tc.tile_pool(name="y1p", bufs=1) as ypool,
            tc.tile_pool(name="x2p", bufs=1) as xpool,
            tc.tile_pool(name="zt", bufs=2) as zpool,
            tc.tile_pool(name="stats", bufs=1) as stats_pool,
        ):
            # ---- constants (DVE-local copies so consumers carry <=1 wait) ----
            def dve_const(t_dram, shape, dtp):
                nm = t_dram.name if hasattr(t_dram, "name") else str(t_dram)
                ld = lpool.tile(shape, dtp, tag="ld")
                nc.sync.dma_start(out=ld[:], in_=t_dram[:])
                cp = cpool.tile(shape, dtp, tag=f"const_{nm}")
                nc.vector.tensor_copy(out=cp[:], in_=ld[:])
                return cp

            iota_g = cpool.tile([128, W], dt.float16)
            nc.gpsimd.iota(iota_g[:], pattern=[[1, W]], channel_multiplier=0,
                           allow_small_or_imprecise_dtypes=True)
            iota_sb = cpool.tile([128, W], dt.float16)
            nc.vector.tensor_copy(out=iota_sb[:], in_=iota_g[:])

            w1_sb = dve_const(t_w1, [D, D], dt.float16)
            w2_sb = dve_const(t_w2, [D, D], dt.float16)
            dnnm_sb = dve_const(t_dnnm, [128, NT // 128], dt.float32)
            dnsh_sb = dve_const(t_dnsh, [128, SHP // 128], dt.float32)
            dnd_sb = dve_const(t_dnd, [128, SHP], dt.float16)
            g1_sb = dve_const(t_g1, [128, 1], dt.float32)
            be1_sb = dve_const(t_be1, [128, 1], dt.float32)
            off1_sb = dve_const(t_off1, [128, l1["total_tiles"]], dt.float16)
            off2_sb = dve_const(t_off2, [128, l2["total_tiles"]], dt.float16)
            idx1_sb = dve_const(t_idx1, [128, l1["total_tiles"]], dt.int32)
            idx2_sb = dve_const(t_idx2, [128, l2["total_tiles"]], dt.int32)

            y1_sb = ypool.tile([128, SHP], dt.float32)
            x2_sb = xpool.tile([128, SHP], dt.float16)
            spa1 = stats_pool.tile([128, NSW], dt.float32)   # L1 sum partials
            spb1 = stats_pool.tile([128, NSW], dt.float32)   # L1 sumsq partials
            spa2 = stats_pool.tile([128, NSW], dt.float32)   # L2 sum partials
            spb2 = stats_pool.tile([128, NSW], dt.float32)   # L2 sumsq partials
            tail_sb = stats_pool.tile([128, NM + 2], dt.float32)

            # ================= L1: full table production =================
            # h1n[node, :] = dn_src[node] * (feat_fm[:, node].T @ W1)
            nprod = NT // 128
            for b0 in range(0, nprod, PROD_B):
                bts = min(PROD_B, nprod - b0)
                ft = fpool.tile([128, PROD_B * 128], dt.float16, tag="ft")
                nc.sync.dma_start(out=ft[:, : bts * 128],
                                  in_=t_feat[:, b0 * 128:(b0 + bts) * 128])
                stg = stpool.tile([128, PROD_B * 128], dt.float16, tag="prod_stage")
                for t in range(bts):
                    ti = b0 + t
                    ps = pmpool.tile([128, 128], dt.float32, space="PSUM", tag="prod_ps")
                    nc.tensor.matmul(out=ps[:], lhsT=ft[:, t * 128:(t + 1) * 128],
                                     rhs=w1_sb[:], start=True, stop=True)
                    nc.vector.tensor_tensor(
                        out=stg[:, t * 128:(t + 1) * 128], in0=ps[:],
                        in1=dnnm_sb[:, ti:ti + 1].to_broadcast([128, 128]),
                        op=OP.mult)
                nc.sync.dma_start(
                    out=table1[b0 * 128:(b0 + bts) * 128, :].rearrange(
                        "(b p) d -> p b d", p=128),
                    in_=stg[:, : bts * 128].rearrange("p (b d) -> p b d", d=D))

            # ================= aggregation pass helper =================
            def agg_layer(lx, idx_sb, off_sb, table_t, out_cb):
                tiles = lx["tiles"]
                T = lx["total_tiles"]
                t = 0
                for g in range(NGRP):
                    w0, w1_ = g * WGRP, min((g + 1) * WGRP, NWIN)
                    nbank = (w1_ - w0 + 7) // 8
                    banks = []
                    for bi in range(nbank):
                        ps = ppool.tile([128, SWW], dt.float32, space="PSUM",
                                        tag="agg_ps")
                        nc.vector.memset(ps[:], 0.0)
                        banks.append(ps)
                    last_wd = -1
                    while t < T and tiles[t] < w1_:
                        wd = tiles[t]
                        gt = gpool.tile([128, D], dt.float16, tag="gt")
                        nc.gpsimd.indirect_dma_start(
                            out=gt[:], out_offset=None,
                            in_=table_t[:],
                            in_offset=bass.IndirectOffsetOnAxis(
                                ap=idx_sb[:, t:t + 1], axis=0))
                        st = spool.tile([128, W], dt.float16, tag="s")
                        nc.vector.tensor_tensor(
                            out=st[:], in0=iota_sb[:],
                            in1=off_sb[:, t:t + 1].to_broadcast([128, W]),
                            op=OP.is_equal)
                        bi = (wd - w0) // 8
                        co = ((wd - w0) % 8) * W
                        nc.tensor.matmul(
                            out=banks[bi][:, co:co + W],
                            lhsT=gt[:], rhs=st[:], start=(wd != last_wd),
                            stop=False)
                        last_wd = wd
                        t += 1
                    for bi in range(nbank):
                        out_cb((w0 // 8) + bi, banks[bi])

            # ---- L1 aggregation: y1 = psum * dnd ; stats partials ----
            def l1_out(sw, ps):
                sl = slice(sw * SWW, (sw + 1) * SWW)
                nc.vector.tensor_tensor(out=y1_sb[:, sl], in0=ps[:],
                                        in1=dnd_sb[:, sl], op=OP.mult)
                lim = SWW if sw < NSW - 1 else (SH - (NSW - 1) * SWW)
                nc.vector.reduce_sum(out=spa1[:, sw:sw + 1],
                                     in_=y1_sb[:, sw * SWW: sw * SWW + lim],
                                     axis=mybir.AxisListType.X)
                sq = zpool.tile([128, SWW], dt.float32, tag="sq")
                nc.scalar.activation(out=sq[:, :lim],
                                     in_=y1_sb[:, sw * SWW: sw * SWW + lim],
                                     func=AF.Square,
                                     accum_out=spb1[:, sw:sw + 1])

            agg_layer(l1, idx1_sb, off1_sb, table1, l1_out)

            # ---- BN1 stats allreduce ----
            stat_c = stats_pool.tile([128, 2], dt.float32)
            nc.vector.reduce_sum(out=stat_c[:, 0:1], in_=spa1[:],
                                 axis=mybir.AxisListType.X)
            nc.vector.reduce_sum(out=stat_c[:, 1:2], in_=spb1[:],
                                 axis=mybir.AxisListType.X)
            nc.sync.dma_start(out=st_in[:], in_=stat_c[:])
            nc.gpsimd.collective_compute(
                "AllReduce", mybir.AluOpType.add,
                replica_groups=[list(range(C))],
                ins=[st_in[:]], outs=[st_out[:]],
            )
            stat_ld = lpool.tile([128, 2], dt.float32, tag="statld")
            nc.sync.dma_start(out=stat_ld[:], in_=st_out[:])
            stat_t = stats_pool.tile([128, 2], dt.float32)
            nc.vector.tensor_copy(out=stat_t[:], in_=stat_ld[:])

            # mu = sum/N ; var = sumsq/N - mu^2 ; s = g1 / sqrt(var+eps)
            # t = be1 - mu * s
            mu = stats_pool.tile([128, 1], dt.float32)
            nc.vector.tensor_scalar_mul(mu[:], stat_t[:, 0:1], 1.0 / N)
            var = stats_pool.tile([128, 1], dt.float32)
            nc.vector.tensor_scalar_mul(var[:], stat_t[:, 1:2], 1.0 / N)
            mu2 = stats_pool.tile([128, 1], dt.float32)
            nc.vector.tensor_tensor(out=mu2[:], in0=mu[:], in1=mu[:], op=OP.mult)
            nc.vector.tensor_tensor(out=var[:], in0=var[:], in1=mu2[:], op=OP.subtract)
            nc.vector.tensor_scalar_add(var[:], var[:], BN_EPS)
            sd = stats_pool.tile([128, 1], dt.float32)
            nc.scalar.activation(out=sd[:], in_=var[:], func=AF.Sqrt)
            rsd = stats_pool.tile([128, 1], dt.float32)
            nc.vector.reciprocal(out=rsd[:], in_=sd[:])
            # one Newton step on r ~= 1/sqrt(v): r <- r*(1.5 - 0.5*v*r^2)
            nt1 = stats_pool.tile([128, 1], dt.float32)
            nc.vector.tensor_tensor(out=nt1[:], in0=rsd[:], in1=rsd[:], op=OP.mult)
            nc.vector.tensor_tensor(out=nt1[:], in0=nt1[:], in1=var[:], op=OP.mult)
            nc.vector.tensor_scalar(out=nt1[:], in0=nt1[:], scalar1=-0.5,
                                    scalar2=1.5, op0=OP.mult, op1=OP.add)
            nc.vector.tensor_tensor(out=rsd[:], in0=rsd[:], in1=nt1[:], op=OP.mult)
            bs = stats_pool.tile([128, 1], dt.float32)
            nc.vector.tensor_tensor(out=bs[:], in0=g1_sb[:], in1=rsd[:], op=OP.mult)
            bt = stats_pool.tile([128, 1], dt.float32)
            nc.vector.tensor_tensor(out=bt[:], in0=mu[:], in1=bs[:], op=OP.mult)
            nc.vector.tensor_tensor(out=bt[:], in0=be1_sb[:], in1=bt[:], op=OP.subtract)

            # ---- x2 = prelu(y1 * s + t) via ACT LeakyRelu ----
            for sw in range(NSW):
                sl = slice(sw * SWW, (sw + 1) * SWW)
                nc.scalar.activation(out=x2_sb[:, sl], in_=y1_sb[:, sl],
                                     func=AF.Lrelu, scale=bs[:, 0:1],
                                     bias=bt[:, 0:1], alpha=a1)

            # ---- L2 shard table production + exchange ----
            nprod2 = SHP // 128
            for b0 in range(0, nprod2, PROD_B):
                bts = min(PROD_B, nprod2 - b0)
                stg = stpool.tile([128, PROD_B * 128], dt.float16, tag="prod_stage")
                for t in range(bts):
                    ti = b0 + t
                    ps = pmpool.tile([128, 128], dt.float32, space="PSUM", tag="prod_ps")
                    nc.tensor.matmul(out=ps[:], lhsT=x2_sb[:, ti * 128:(ti + 1) * 128],
                                     rhs=w2_sb[:], start=True, stop=True)
                    nc.vector.tensor_tensor(
                        out=stg[:, t * 128:(t + 1) * 128], in0=ps[:],
                        in1=dnsh_sb[:, ti:ti + 1].to_broadcast([128, 128]),
                        op=OP.mult)
                nc.sync.dma_start(
                    out=bounce[b0 * 128:(b0 + bts) * 128, :].rearrange(
                        "(b p) d -> p b d", p=128),
                    in_=stg[:, : bts * 128].rearrange("p (b d) -> p b d", d=D))
            nc.gpsimd.collective_compute(
                "AllGather", mybir.AluOpType.bypass,
                replica_groups=[list(range(C))],
                ins=[bounce[:]], outs=[table2[:]],
            )

            # ---- L2 aggregation: stats partials + mask columns only ----
            def l2_out(sw, ps):
                z = zpool.tile([128, SWW], dt.float32, tag="z")
                sl = slice(sw * SWW, (sw + 1) * SWW)
                nc.vector.tensor_tensor(out=z[:], in0=ps[:], in1=dnd_sb[:, sl],
                                        op=OP.mult)
                lim = SWW if sw < NSW - 1 else (SH - (NSW - 1) * SWW)
                nc.vector.reduce_sum(out=spa2[:, sw:sw + 1], in_=z[:, :lim],
                                     axis=mybir.AxisListType.X)
                sq = zpool.tile([128, SWW], dt.float32, tag="sq")
                nc.scalar.activation(out=sq[:, :lim], in_=z[:, :lim],
                                     func=AF.Square,
                                     accum_out=spb2[:, sw:sw + 1])
                if sw == NSW - 1:
                    nc.vector.tensor_copy(out=tail_sb[:, 0:NM],
                                          in_=z[:, lim: lim + NM])

            agg_layer(l2, idx2_sb, off2_sb, table2, l2_out)

            nc.vector.reduce_sum(out=tail_sb[:, NM:NM + 1], in_=spa2[:],
                                 axis=mybir.AxisListType.X)
            nc.vector.reduce_sum(out=tail_sb[:, NM + 1:NM + 2], in_=spb2[:],
                                 axis=mybir.AxisListType.X)
            nc.sync.dma_start(out=t_tail[:], in_=tail_sb[:])

    return nc


def _legalize_waits(nc):
    """This toolchain's walrus accepts only ONE sync-wait command per
    instruction; Tile emits up to ~3. Split excess waits onto same-engine
    NoOps placed immediately before (engine program order preserves
    semantics)."""
    import concourse.mybir as mybir
    f = nc.m.functions[0]
    k = 0
    for b in f.blocks:
        il = b.instructions
        out = []
        changed = False
        for inst in il:
            si = inst.sync_info
            nw = list(si.on_wait) if si is not None else []
            if len(nw) > 1:
                for w in nw[:-1]:
                    nop = mybir.InstNoOp(name=f"I-wfix-{k}", ins=[], outs=[])
                    k += 1
                    nop.engine = inst.engine
                    nop.sync_info = mybir.SyncInfo(on_wait=[w], on_update=[])
                    out.append(nop)
                si.on_wait = nw[-1:]
                changed = True
            out.append(inst)
        if changed:
            b.instructions = out
    return k


def _host_target_and_loss(inputs, tail_results):
    """Target branch + final loss, replicating the reference's jax-CPU f32
    arithmetic exactly (the target mean is BN-forced to ~0, so its direction is
    rounding noise that must be reproduced bit-for-bit)."""
    import jax
    cpu = jax.devices("cpu")[0]
    import jax.numpy as jnp

    with jax.default_device(cpu):
        def _deg_norm(idx, n):
            deg = jax.ops.segment_sum(jnp.ones(idx.shape[0], jnp.float32), idx,
                                      num_segments=n)
            return jnp.where(deg > 0, deg ** -0.5, 0.0)

        def graph_conv(x, Wm, b, s_, d_, n):
            h = x @ Wm
            h = h * _deg_norm(s_, n)[:, None]
            agg = jax.ops.segment_sum(h[s_], d_, num_segments=n)
            return agg * _deg_norm(d_, n)[:, None] + b

        def batch_norm(x, g, be):
            mu = x.mean(axis=0)
            var = ((x - mu) ** 2).mean(axis=0)
            return (x - mu) * jax.lax.rsqrt(var + BN_EPS) * g + be

        def prelu(x, a):
            return jnp.where(x >= 0, x, a * x)

        def encoder(x, s_, d_, n, p):
            (W1, b1, g1, be1, a1, W2, b2, g2, be2, a2) = p
            h = graph_conv(x, W1, b1, s_, d_, n)
            h = prelu(batch_norm(h, g1, be1), a1)
            h = graph_conv(h, W2, b2, s_, d_, n)
            h = batch_norm(h, g2, be2)
            return h

        def sce_loss(x, y):
            xn = x / jnp.maximum(jnp.linalg.norm(x), 1e-12)
            yn = y / jnp.maximum(jnp.linalg.norm(y), 1e-12)
            return 1.0 - jnp.sum(xn * yn)

        feat = jnp.asarray(inputs["feat"], jnp.float32)
        mask = jnp.asarray(inputs["mask_nodes"], jnp.int32)
        pt = tuple(jnp.asarray(inputs[nm + "t"]) for nm in
                   ["W1", "b1", "g1", "be1", "a1", "W2", "b2", "g2", "be2", "a2"])
        sub_feat = feat[mask]
        h2 = encoder(sub_feat, jnp.asarray(inputs["sub_src"], jnp.int32),
                     jnp.asarray(inputs["sub_dst"], jnp.int32), NM, pt)
        bvec = h2.mean(axis=0)

        # combine device partials -> a vector
        sums = np.zeros((128,), np.float32)
        sumsq = np.zeros((128,), np.float32)
        for r in tail_results:
            sums += r[:, NM]
            sumsq += r[:, NM + 1]
        mu2 = sums / np.float32(N)
        var2 = sumsq / np.float32(N) - mu2 * mu2
        g2 = np.asarray(inputs["g2o"], np.float32)
        be2 = np.asarray(inputs["be2o"], np.float32)
        s2 = g2 / np.sqrt(var2 + np.float32(BN_EPS))
        cols = tail_results[0][:, :NM]           # [128 feat, 17]
        hbn = (cols - mu2[:, None]) * s2[:, None] + be2[:, None]
        avec = hbn.mean(axis=1)

        loss = sce_loss(jnp.asarray(avec), bvec)
        return np.asarray(loss, np.float32)


def kernel(**inputs):
    from concourse.bass_utils import run_bass_kernel_spmd

    prep = _host_prep(inputs)
    nc = _build(prep, inputs)

    w1 = np.ascontiguousarray(np.asarray(inputs["W1o"], np.float32).astype(np.float16))
    w2 = np.ascontiguousarray(np.asarray(inputs["W2o"], np.float32).astype(np.float16))
    g1 = np.asarray(inputs["g1o"], np.float32).reshape(128, 1)
    be1 = np.asarray(inputs["be1o"], np.float32).reshape(128, 1)

    in_maps = []
    for c in range(C):
        in_maps.append({
            "feat_fm": prep["feat_fm"],
            "w1": w1, "w2": w2,
            "dn_nm": prep["dn_nm"],
            "dn_sh": prep["dn_sh"][c],
            "dnd_rep": prep["dnd_rep"][c],
            "g1": g1, "be1": be1,
            "idx1": prep["l1"]["idx32"][c],
            "idx2": prep["l2"]["idx32"][c],
            "off1": prep["l1"]["offs"][c],
            "off2": prep["l2"]["offs"][c],
        })
    _legalize_waits(nc)
    res = run_bass_kernel_spmd(nc, in_maps, core_ids=list(range(C)))
    tails = [res.results[c]["tail"] for c in range(C)]
    return _host_target_and_loss(inputs, tails)


# revision 19
# speedup vs baseline: 1.1507x; 1.0786x over previous
"""Trainium2 Bass kernel for the GraphMAE-style GCN loss (nn_CG_30588757082611).

Strategy (8 NeuronCores, SPMD):
  - Nodes sharded 12500/core (padded to 12800); edges partitioned by dst owner.
  - Layer tables (h @ W, pre-scaled by src-degree norm) stored node-major fp16
    in DRAM; edge gathers via gpsimd.indirect_dma_start (128 rows/call).
  - Aggregation: one-hot matmuls (lhsT = gathered edge rows, rhs = iota==offset
    selection matrix) accumulated feature-major in PSUM per 512-dst bank.
  - BN statistics via per-feature free-dim reductions; BN1 stats all-reduced
    on-device (tiny collective); layer-2 hidden shards exchanged via AllGather.
  - The 17-node target branch mean is mathematically zero (BN over the same
    rows), so its normalized direction is pure f32 rounding noise; it is
    computed bit-identically to the reference with jax on CPU. The main branch
    (everything O(N·E)) runs on the NeuronCores.
"""

import sys

sys.path.insert(0, "/opt/trn_rl_repo")

import numpy as np

N = 100000
D = 128
C = 8
SH = 12500          # real nodes per core
SHP = 12800         # padded nodes per core
NT = C * SHP        # padded table rows (102400)
W = 512             # dst window width (S matrix / psum slice)
NWIN = SHP // W     # 25 windows per core
VWIN = 24           # window holding the virtual mask columns
SWW = 512           # psum bank width (1 window)
NSW = SHP // SWW    # 25 super windows
WPB = SWW // W      # windows per psum bank
WGRP = 4            # windows per psum group
NGRP = (NWIN + WGRP - 1) // WGRP  # 7 groups (last short)
NM = 17
BN_EPS = 1e-5
SENT_OFF = np.float16(1000.0)

_cached = {}


def _pad_id(n):
    return (n // SH) * SHP + (n % SH)


def _prep_layer_edges(src_pad, dst_core, dst_local, extra_src_pad=None, extra_off=None):
    """Per-core gather/matmul schedule for one layer (no index chunking;
    indirect DMA uses int32 indices). Tiles of 128 edges, window-pure; per
    (core, window) cell padded to the max tile count over cores."""
    cell_src = [[None] * NWIN for _ in range(C)]
    cell_off = [[None] * NWIN for _ in range(C)]
    order = np.lexsort((src_pad, dst_local, dst_core))
    s_s, s_c, s_l = src_pad[order], dst_core[order], dst_local[order]
    s_w = s_l // W
    s_o = (s_l % W).astype(np.float16)
    key = s_c.astype(np.int64) * NWIN + s_w
    uniq, starts = np.unique(key, return_index=True)
    starts = list(starts) + [len(key)]
    for i, k in enumerate(uniq):
        wd = int(k % NWIN)
        co = int(k // NWIN)
        a, b = starts[i], starts[i + 1]
        cell_src[co][wd] = s_s[a:b]
        cell_off[co][wd] = s_o[a:b]
    if extra_src_pad is not None:
        for co in range(C):
            wd = VWIN
            cs = cell_src[co][wd]
            cell_src[co][wd] = (extra_src_pad if cs is None
                                else np.concatenate([cs, extra_src_pad]))
            cc = cell_off[co][wd]
            eo = extra_off.astype(np.float16)
            cell_off[co][wd] = eo if cc is None else np.concatenate([cc, eo])

    tiles = []          # per tile: window id
    idx_cols = [[] for _ in range(C)]
    offs_cols = [[] for _ in range(C)]
    for wd in range(NWIN):
        cnts = [0 if cell_src[co][wd] is None else len(cell_src[co][wd])
                for co in range(C)]
        nt = (max(cnts) + 127) // 128
        for co in range(C):
            ns = nt * 128
            sp = np.zeros(ns, np.int64)
            op = np.full(ns, SENT_OFF, np.float16)
            if cell_src[co][wd] is not None:
                cs = cell_src[co][wd]
                sp[: len(cs)] = cs
                op[: len(cs)] = cell_off[co][wd]
            idx_cols[co].append(sp.astype(np.int32))
            offs_cols[co].append(op)
        tiles.extend([wd] * nt)
    total_tiles = len(tiles)
    idx32 = []
    offs = []
    for co in range(C):
        flat = np.concatenate(idx_cols[co]).reshape(total_tiles, 128).T
        idx32.append(np.ascontiguousarray(flat))               # [128, T]
        ob = np.concatenate(offs_cols[co]).reshape(total_tiles, 128).T
        offs.append(np.ascontiguousarray(ob))                  # [128, T]
    return dict(tiles=tiles, idx32=idx32, offs=offs, total_tiles=total_tiles)


def _host_prep(inputs):
    feat = np.asarray(inputs["feat"], np.float32)
    src = np.asarray(inputs["src"], np.int64)
    dst = np.asarray(inputs["dst"], np.int64)
    mask = np.asarray(inputs["mask_nodes"], np.int64)

    deg_out = np.bincount(src, minlength=N).astype(np.float32)
    deg_in = np.bincount(dst, minlength=N).astype(np.float32)
    dns = np.where(deg_out > 0, deg_out ** -0.5, 0.0).astype(np.float32)
    dnd = np.where(deg_in > 0, deg_in ** -0.5, 0.0).astype(np.float32)

    src_pad = _pad_id(src)
    dst_core = dst // SH
    dst_local = dst % SH

    l1 = _prep_layer_edges(src_pad, dst_core, dst_local)
    # layer 2 adds replicated "virtual" copies of mask-node in-edges, landing at
    # window 24 offsets 212..228 (local cols 12500..12516)
    vsrc, voff = [], []
    for j, m in enumerate(mask):
        sel = dst == m
        vsrc.append(src_pad[sel])
        voff.append(np.full(sel.sum(), 212 + j, np.float16))
    vsrc = np.concatenate(vsrc)
    voff = np.concatenate(voff)
    l2 = _prep_layer_edges(src_pad, dst_core, dst_local, vsrc, voff)

    # feature-major padded masked features (fp16), replicated to all cores
    featm = feat.copy()
    featm[mask] = 0.0
    feat_fm = np.zeros((D, NT), np.float16)
    for c in range(C):
        feat_fm[:, c * SHP: c * SHP + SH] = featm[c * SH:(c + 1) * SH].T
    # node-major dn_src per production tile (full padded table), [128, NT//128]
    dns_pad = np.zeros(NT, np.float32)
    for c in range(C):
        dns_pad[c * SHP: c * SHP + SH] = dns[c * SH:(c + 1) * SH]
    dn_nm = np.ascontiguousarray(dns_pad.reshape(NT // 128, 128).T)  # [128, 800]
    # per-core shard dn_src node-major [128, 100]
    dn_sh = []
    for c in range(C):
        v = dns_pad[c * SHP:(c + 1) * SHP]
        dn_sh.append(np.ascontiguousarray(v.reshape(SHP // 128, 128).T))
    # per-core dst-degree norm replicated across partitions [128, SHP] fp16,
    # with virtual columns = mask nodes' dnd
    dnd_rep = []
    for c in range(C):
        v = np.zeros(SHP, np.float32)
        v[:SH] = dnd[c * SH:(c + 1) * SH]
        v[SH: SH + NM] = dnd[mask]
        dnd_rep.append(np.tile(v.astype(np.float16)[None, :], (128, 1)))
    return dict(l1=l1, l2=l2, feat_fm=feat_fm, dn_nm=dn_nm, dn_sh=dn_sh,
                dnd_rep=dnd_rep)


def _build(prep, w_np):
    import concourse.bass as bass
    import concourse.mybir as mybir
    import concourse.tile as tile

    dt = mybir.dt
    AF = mybir.ActivationFunctionType
    OP = mybir.AluOpType
    l1, l2 = prep["l1"], prep["l2"]
    a1 = float(w_np["a1o"])

    nc = bass.Bass()
    t_feat = nc.declare_dram_parameter("feat_fm", [D, NT], dt.float16, isOutput=False)
    t_w1 = nc.declare_dram_parameter("w1", [D, D], dt.float16, isOutput=False)
    t_w2 = nc.declare_dram_parameter("w2", [D, D], dt.float16, isOutput=False)
    t_dnnm = nc.declare_dram_parameter("dn_nm", [128, NT // 128], dt.float32, isOutput=False)
    t_dnsh = nc.declare_dram_parameter("dn_sh", [128, SHP // 128], dt.float32, isOutput=False)
    t_dnd = nc.declare_dram_parameter("dnd_rep", [128, SHP], dt.float16, isOutput=False)
    t_g1 = nc.declare_dram_parameter("g1", [128, 1], dt.float32, isOutput=False)
    t_be1 = nc.declare_dram_parameter("be1", [128, 1], dt.float32, isOutput=False)
    t_idx1 = nc.declare_dram_parameter("idx1", [128, l1["total_tiles"]], dt.int32, isOutput=False)
    t_idx2 = nc.declare_dram_parameter("idx2", [128, l2["total_tiles"]], dt.int32, isOutput=False)
    t_off1 = nc.declare_dram_parameter("off1", [128, l1["total_tiles"]], dt.float16, isOutput=False)
    t_off2 = nc.declare_dram_parameter("off2", [128, l2["total_tiles"]], dt.float16, isOutput=False)
    t_tail = nc.declare_dram_parameter("tail", [128, NM + 2], dt.float32, isOutput=True)

    table1 = nc.dram_tensor("table1", [NT, D], dt.float16)
    bounce = nc.dram_tensor("bounce", [SHP, D], dt.float16)
    table2 = nc.dram_tensor("table2", [NT, D], dt.float16, addr_space="Shared")
    st_in = nc.dram_tensor("st_in", [128, 2], dt.float32)
    st_out = nc.dram_tensor("st_out", [128, 2], dt.float32, addr_space="Shared")

    PROD_B = 16  # production tiles per staged table write

    with tile.TileContext(nc) as tc:
        with (
            tc.tile_pool(name="const", bufs=1) as cpool,
            tc.tile_pool(name="ld", bufs=2) as lpool,
            tc.tile_pool(name="gath", bufs=8) as gpool,
            tc.tile_pool(name="idxp", bufs=3) as ipool,
            tc.tile_pool(name="sp", bufs=4) as spool,
            tc.tile_pool(name="feat", bufs=2) as fpool,
            tc.tile_pool(name="stage", bufs=2) as stpool,
            tc.tile_pool(name="psum", bufs=4, space="PSUM") as ppool,
            tc.tile_pool(name="psmm", bufs=2, space="PSUM") as pmpool,
            tc.tile_pool(name="y1p", bufs=1) as ypool,
            tc.tile_pool(name="x2p", bufs=1) as xpool,
            tc.tile_pool(name="zt", bufs=2) as zpool,
            tc.tile_pool(name="stats", bufs=1) as stats_pool,
        ):
            # ---- constants (DVE-local copies so consumers carry <=1 wait) ----
            def dve_const(t_dram, shape, dtp):
                nm = t_dram.name if hasattr(t_dram, "name") else str(t_dram)
                ld = lpool.tile(shape, dtp, tag="ld")
                nc.sync.dma_start(out=ld[:], in_=t_dram[:])
                cp = cpool.tile(shape, dtp, tag=f"const_{nm}")
                nc.vector.tensor_copy(out=cp[:], in_=ld[:])
                return cp

            iota_g = cpool.tile([128, W], dt.float16)
            nc.gpsimd.iota(iota_g[:], pattern=[[1, W]], channel_multiplier=0,
                           allow_small_or_imprecise_dtypes=True)
            iota_sb = cpool.tile([128, W], dt.float16)
            nc.vector.tensor_copy(out=iota_sb[:], in_=iota_g[:])

            w1_sb = dve_const(t_w1, [D, D], dt.float16)
            w2_sb = dve_const(t_w2, [D, D], dt.float16)
            dnnm_sb = dve_const(t_dnnm, [128, NT // 128], dt.float32)
            dnsh_sb = dve_const(t_dnsh, [128, SHP // 128], dt.float32)
            dnd_sb = dve_const(t_dnd, [128, SHP], dt.float16)
            g1_sb = dve_const(t_g1, [128, 1], dt.float32)
            be1_sb = dve_const(t_be1, [128, 1], dt.float32)
            off1_sb = dve_const(t_off1, [128, l1["total_tiles"]], dt.float16)
            off2_sb = dve_const(t_off2, [128, l2["total_tiles"]], dt.float16)
            idx1_sb = dve_const(t_idx1, [128, l1["total_tiles"]], dt.int32)
            idx2_sb = dve_const(t_idx2, [128, l2["total_tiles"]], dt.int32)

            y1_sb = ypool.tile([128, SHP], dt.float32)
            x2_sb = xpool.tile([128, SHP], dt.float16)
            spa1 = stats_pool.tile([128, NSW], dt.float32)   # L1 sum partials
            spb1 = stats_pool.tile([128, NSW], dt.float32)   # L1 sumsq partials
            spa2 = stats_pool.tile([128, NSW], dt.float32)   # L2 sum partials
            spb2 = stats_pool.tile([128, NSW], dt.float32)   # L2 sumsq partials
            tail_sb = stats_pool.tile([128, NM + 2], dt.float32)

            # ================= L1: full table production =================
            # h1n[node, :] = dn_src[node] * (feat_fm[:, node].T @ W1)
            nprod = NT // 128
            for b0 in range(0, nprod, PROD_B):
                bts = min(PROD_B, nprod - b0)
                ft = fpool.tile([128, PROD_B * 128], dt.float16, tag="ft")
                nc.sync.dma_start(out=ft[:, : bts * 128],
                                  in_=t_feat[:, b0 * 128:(b0 + bts) * 128])
                stg = stpool.tile([128, PROD_B * 128], dt.float16, tag="prod_stage")
                for t in range(bts):
                    ti = b0 + t
                    ps = pmpool.tile([128, 128], dt.float32, space="PSUM", tag="prod_ps")
                    nc.tensor.matmul(out=ps[:], lhsT=ft[:, t * 128:(t + 1) * 128],
                                     rhs=w1_sb[:], start=True, stop=True)
                    nc.vector.tensor_tensor(
                        out=stg[:, t * 128:(t + 1) * 128], in0=ps[:],
                        in1=dnnm_sb[:, ti:ti + 1].to_broadcast([128, 128]),
                        op=OP.mult)
                nc.sync.dma_start(
                    out=table1[b0 * 128:(b0 + bts) * 128, :].rearrange(
                        "(b p) d -> p b d", p=128),
                    in_=stg[:, : bts * 128].rearrange("p (b d) -> p b d", d=D))

            # ================= aggregation pass helper =================
            def agg_layer(lx, idx_sb, off_sb, table_t, out_cb):
                tiles = lx["tiles"]
                T = lx["total_tiles"]
                t = 0
                for g in range(NGRP):
                    w0, w1_ = g * WGRP, min((g + 1) * WGRP, NWIN)
                    nbank = (w1_ - w0 + WPB - 1) // WPB
                    banks = []
                    for bi in range(nbank):
                        ps = ppool.tile([128, SWW], dt.float32, space="PSUM",
                                        tag="agg_ps")
                        nc.vector.memset(ps[:], 0.0)
                        banks.append(ps)
                    last_wd = -1
                    while t < T and tiles[t] < w1_:
                        wd = tiles[t]
                        gt = gpool.tile([128, D], dt.float16, tag="gt")
                        nc.gpsimd.indirect_dma_start(
                            out=gt[:], out_offset=None,
                            in_=table_t[:],
                            in_offset=bass.IndirectOffsetOnAxis(
                                ap=idx_sb[:, t:t + 1], axis=0))
                        st = spool.tile([128, W], dt.float16, tag="s")
                        nc.vector.tensor_tensor(
                            out=st[:], in0=iota_sb[:],
                            in1=off_sb[:, t:t + 1].to_broadcast([128, W]),
                            op=OP.is_equal)
                        bi = (wd - w0) // WPB
                        co = ((wd - w0) % WPB) * W
                        nc.tensor.matmul(
                            out=banks[bi][:, co:co + W],
                            lhsT=gt[:], rhs=st[:], start=(wd != last_wd),
                            stop=False)
                        last_wd = wd
                        t += 1
                    for bi in range(nbank):
                        out_cb((w0 * W) // SWW + bi, banks[bi])

            # ---- L1 aggregation: y1 = psum * dnd ; stats partials ----
            def l1_out(sw, ps):
                sl = slice(sw * SWW, (sw + 1) * SWW)
                nc.vector.tensor_tensor(out=y1_sb[:, sl], in0=ps[:],
                                        in1=dnd_sb[:, sl], op=OP.mult)
                lim = SWW if sw < NSW - 1 else (SH - (NSW - 1) * SWW)
                nc.vector.reduce_sum(out=spa1[:, sw:sw + 1],
                                     in_=y1_sb[:, sw * SWW: sw * SWW + lim],
                                     axis=mybir.AxisListType.X)
                sq = zpool.tile([128, SWW], dt.float32, tag="sq")
                nc.scalar.activation(out=sq[:, :lim],
                                     in_=y1_sb[:, sw * SWW: sw * SWW + lim],
                                     func=AF.Square,
                                     accum_out=spb1[:, sw:sw + 1])

            agg_layer(l1, idx1_sb, off1_sb, table1, l1_out)

            # ---- BN1 stats allreduce ----
            stat_c = stats_pool.tile([128, 2], dt.float32)
            nc.vector.reduce_sum(out=stat_c[:, 0:1], in_=spa1[:],
                                 axis=mybir.AxisListType.X)
            nc.vector.reduce_sum(out=stat_c[:, 1:2], in_=spb1[:],
                                 axis=mybir.AxisListType.X)
            nc.sync.dma_start(out=st_in[:], in_=stat_c[:])
            nc.gpsimd.collective_compute(
                "AllReduce", mybir.AluOpType.add,
                replica_groups=[list(range(C))],
                ins=[st_in[:]], outs=[st_out[:]],
            )
            stat_ld = lpool.tile([128, 2], dt.float32, tag="statld")
            nc.sync.dma_start(out=stat_ld[:], in_=st_out[:])
            stat_t = stats_pool.tile([128, 2], dt.float32)
            nc.vector.tensor_copy(out=stat_t[:], in_=stat_ld[:])

            # mu = sum/N ; var = sumsq/N - mu^2 ; s = g1 / sqrt(var+eps)
            # t = be1 - mu * s
            mu = stats_pool.tile([128, 1], dt.float32)
            nc.vector.tensor_scalar_mul(mu[:], stat_t[:, 0:1], 1.0 / N)
            var = stats_pool.tile([128, 1], dt.float32)
            nc.vector.tensor_scalar_mul(var[:], stat_t[:, 1:2], 1.0 / N)
            mu2 = stats_pool.tile([128, 1], dt.float32)
            nc.vector.tensor_tensor(out=mu2[:], in0=mu[:], in1=mu[:], op=OP.mult)
            nc.vector.tensor_tensor(out=var[:], in0=var[:], in1=mu2[:], op=OP.subtract)
            nc.vector.tensor_scalar_add(var[:], var[:], BN_EPS)
            sd = stats_pool.tile([128, 1], dt.float32)
            nc.scalar.activation(out=sd[:], in_=var[:], func=AF.Sqrt)
            rsd = stats_pool.tile([128, 1], dt.float32)
            nc.vector.reciprocal(out=rsd[:], in_=sd[:])
            # one Newton step on r ~= 1/sqrt(v): r <- r*(1.5 - 0.5*v*r^2)
            nt1 = stats_pool.tile([128, 1], dt.float32)
            nc.vector.tensor_tensor(out=nt1[:], in0=rsd[:], in1=rsd[:], op=OP.mult)
            nc.vector.tensor_tensor(out=nt1[:], in0=nt1[:], in1=var[:], op=OP.mult)
            nc.vector.tensor_scalar(out=nt1[:], in0=nt1[:], scalar1=-0.5,
                                    scalar2=1.5, op0=OP.mult, op1=OP.add)
            nc.vector.tensor_tensor(out=rsd[:], in0=rsd[:], in1=nt1[:], op=OP.mult)
            bs = stats_pool.tile([128, 1], dt.float32)
            nc.vector.tensor_tensor(out=bs[:], in0=g1_sb[:], in1=rsd[:], op=OP.mult)
            bt = stats_pool.tile([128, 1], dt.float32)
            nc.vector.tensor_tensor(out=bt[:], in0=mu[:], in1=bs[:], op=OP.mult)
            nc.vector.tensor_tensor(out=bt[:], in0=be1_sb[:], in1=bt[:], op=OP.subtract)

            # ---- x2 = prelu(y1 * s + t) via ACT LeakyRelu ----
            for sw in range(NSW):
                sl = slice(sw * SWW, (sw + 1) * SWW)
                nc.scalar.activation(out=x2_sb[:, sl], in_=y1_sb[:, sl],
                                     func=AF.Lrelu, scale=bs[:, 0:1],
                                     bias=bt[:, 0:1], alpha=a1)

            # ---- L2 shard table production + exchange ----
            nprod2 = SHP // 128
            for b0 in range(0, nprod2, PROD_B):
                bts = min(PROD_B, nprod2 - b0)
                stg = stpool.tile([128, PROD_B * 128], dt.float16, tag="prod_stage")
                for t in range(bts):
                    ti = b0 + t
                    ps = pmpool.tile([128, 128], dt.float32, space="PSUM", tag="prod_ps")
                    nc.tensor.matmul(out=ps[:], lhsT=x2_sb[:, ti * 128:(ti + 1) * 128],
                                     rhs=w2_sb[:], start=True, stop=True)
                    nc.vector.tensor_tensor(
                        out=stg[:, t * 128:(t + 1) * 128], in0=ps[:],
                        in1=dnsh_sb[:, ti:ti + 1].to_broadcast([128, 128]),
                        op=OP.mult)
                nc.sync.dma_start(
                    out=bounce[b0 * 128:(b0 + bts) * 128, :].rearrange(
                        "(b p) d -> p b d", p=128),
                    in_=stg[:, : bts * 128].rearrange("p (b d) -> p b d", d=D))
            nc.gpsimd.collective_compute(
                "AllGather", mybir.AluOpType.bypass,
                replica_groups=[list(range(C))],
                ins=[bounce[:]], outs=[table2[:]],
            )

            # ---- L2 aggregation: stats partials + mask columns only ----
            def l2_out(sw, ps):
                z = zpool.tile([128, SWW], dt.float32, tag="z")
                sl = slice(sw * SWW, (sw + 1) * SWW)
                nc.vector.tensor_tensor(out=z[:], in0=ps[:], in1=dnd_sb[:, sl],
                                        op=OP.mult)
                lim = SWW if sw < NSW - 1 else (SH - (NSW - 1) * SWW)
                nc.vector.reduce_sum(out=spa2[:, sw:sw + 1], in_=z[:, :lim],
                                     axis=mybir.AxisListType.X)
                sq = zpool.tile([128, SWW], dt.float32, tag="sq")
                nc.scalar.activation(out=sq[:, :lim], in_=z[:, :lim],
                                     func=AF.Square,
                                     accum_out=spb2[:, sw:sw + 1])
                if sw == NSW - 1:
                    nc.vector.tensor_copy(out=tail_sb[:, 0:NM],
                                          in_=z[:, lim: lim + NM])

            agg_layer(l2, idx2_sb, off2_sb, table2, l2_out)

            nc.vector.reduce_sum(out=tail_sb[:, NM:NM + 1], in_=spa2[:],
                                 axis=mybir.AxisListType.X)
            nc.vector.reduce_sum(out=tail_sb[:, NM + 1:NM + 2], in_=spb2[:],
                                 axis=mybir.AxisListType.X)
            nc.sync.dma_start(out=t_tail[:], in_=tail_sb[:])

    return nc


def _legalize_waits(nc):
    """This toolchain's walrus accepts only ONE sync-wait command per
    instruction; Tile emits up to ~3. Split excess waits onto same-engine
    NoOps placed immediately before (engine program order preserves
    semantics)."""
    import concourse.mybir as mybir
    f = nc.m.functions[0]
    k = 0
    for b in f.blocks:
        il = b.instructions
        out = []
        changed = False
        for inst in il:
            si = inst.sync_info
            nw = list(si.on_wait) if si is not None else []
            if len(nw) > 1:
                for w in nw[:-1]:
                    nop = mybir.InstNoOp(name=f"I-wfix-{k}", ins=[], outs=[])
                    k += 1
                    nop.engine = inst.engine
                    nop.sync_info = mybir.SyncInfo(on_wait=[w], on_update=[])
                    out.append(nop)
                si.on_wait = nw[-1:]
                changed = True
            out.append(inst)
        if changed:
            b.instructions = out
    return k


def _host_target_and_loss(inputs, tail_results):
    """Target branch + final loss, replicating the reference's jax-CPU f32
    arithmetic exactly (the target mean is BN-forced to ~0, so its direction is
    rounding noise that must be reproduced bit-for-bit)."""
    import jax
    cpu = jax.devices("cpu")[0]
    import jax.numpy as jnp

    with jax.default_device(cpu):
        def _deg_norm(idx, n):
            deg = jax.ops.segment_sum(jnp.ones(idx.shape[0], jnp.float32), idx,
                                      num_segments=n)
            return jnp.where(deg > 0, deg ** -0.5, 0.0)

        def graph_conv(x, Wm, b, s_, d_, n):
            h = x @ Wm
            h = h * _deg_norm(s_, n)[:, None]
            agg = jax.ops.segment_sum(h[s_], d_, num_segments=n)
            return agg * _deg_norm(d_, n)[:, None] + b

        def batch_norm(x, g, be):
            mu = x.mean(axis=0)
            var = ((x - mu) ** 2).mean(axis=0)
            return (x - mu) * jax.lax.rsqrt(var + BN_EPS) * g + be

        def prelu(x, a):
            return jnp.where(x >= 0, x, a * x)

        def encoder(x, s_, d_, n, p):
            (W1, b1, g1, be1, a1, W2, b2, g2, be2, a2) = p
            h = graph_conv(x, W1, b1, s_, d_, n)
            h = prelu(batch_norm(h, g1, be1), a1)
            h = graph_conv(h, W2, b2, s_, d_, n)
            h = batch_norm(h, g2, be2)
            return h

        def sce_loss(x, y):
            xn = x / jnp.maximum(jnp.linalg.norm(x), 1e-12)
            yn = y / jnp.maximum(jnp.linalg.norm(y), 1e-12)
            return 1.0 - jnp.sum(xn * yn)

        feat = jnp.asarray(inputs["feat"], jnp.float32)
        mask = jnp.asarray(inputs["mask_nodes"], jnp.int32)
        pt = tuple(jnp.asarray(inputs[nm + "t"]) for nm in
                   ["W1", "b1", "g1", "be1", "a1", "W2", "b2", "g2", "be2", "a2"])
        sub_feat = feat[mask]
        h2 = encoder(sub_feat, jnp.asarray(inputs["sub_src"], jnp.int32),
                     jnp.asarray(inputs["sub_dst"], jnp.int32), NM, pt)
        bvec = h2.mean(axis=0)

        # combine device partials -> a vector
        sums = np.zeros((128,), np.float32)
        sumsq = np.zeros((128,), np.float32)
        for r in tail_results:
            sums += r[:, NM]
            sumsq += r[:, NM + 1]
        mu2 = sums / np.float32(N)
        var2 = sumsq / np.float32(N) - mu2 * mu2
        g2 = np.asarray(inputs["g2o"], np.float32)
        be2 = np.asarray(inputs["be2o"], np.float32)
        s2 = g2 / np.sqrt(var2 + np.float32(BN_EPS))
        cols = tail_results[0][:, :NM]           # [128 feat, 17]
        hbn = (cols - mu2[:, None]) * s2[:, None] + be2[:, None]
        avec = hbn.mean(axis=1)

        loss = sce_loss(jnp.asarray(avec), bvec)
        return np.asarray(loss, np.float32)


def kernel(**inputs):
    from concourse.bass_utils import run_bass_kernel_spmd

    prep = _host_prep(inputs)
    nc = _build(prep, inputs)

    w1 = np.ascontiguousarray(np.asarray(inputs["W1o"], np.float32).astype(np.float16))
    w2 = np.ascontiguousarray(np.asarray(inputs["W2o"], np.float32).astype(np.float16))
    g1 = np.asarray(inputs["g1o"], np.float32).reshape(128, 1)
    be1 = np.asarray(inputs["be1o"], np.float32).reshape(128, 1)

    in_maps = []
    for c in range(C):
        in_maps.append({
            "feat_fm": prep["feat_fm"],
            "w1": w1, "w2": w2,
            "dn_nm": prep["dn_nm"],
            "dn_sh": prep["dn_sh"][c],
            "dnd_rep": prep["dnd_rep"][c],
            "g1": g1, "be1": be1,
            "idx1": prep["l1"]["idx32"][c],
            "idx2": prep["l2"]["idx32"][c],
            "off1": prep["l1"]["offs"][c],
            "off2": prep["l2"]["offs"][c],
        })
    _legalize_waits(nc)
    res = run_bass_kernel_spmd(nc, in_maps, core_ids=list(range(C)))
    tails = [res.results[c]["tail"] for c in range(C)]
    return _host_target_and_loss(inputs, tails)
